# revision 1
# baseline (speedup 1.0000x reference)
"""Multi-head attention kernel for Trainium2 (8 NeuronCores).

Problem: B=2, S=2048, 16 heads, d_head=64, shared 64x64 per-head projections.
  out = softmax((q Wq^T)(k Wk^T)^T / 8) @ (v Wv^T), per (batch, head).

Sharding: 32 (b,h) pairs -> 4 pairs per core (data + head parallel).

Math folding (host):
  scores S = qh (Wq^T Wk / 8) kh^T           -> M1 := Wq^T Wk / 8
  S^T computed on device as khT^T @ (M1^T qhT), so softmax reduction (over k)
  lands on the PSUM partition axis and is absorbed by a ones-column in v:
  O_aug^T = [vh, 1]^T @ exp(S^T)  gives rows 0..63 = (P vh)^T, row 64 = rowsums.
  Epilogue, per 128-row output block: out = O1 @ Wv^T computed directly by the
  PE with the transposed accumulator as lhsT (no staging round-trip), the
  denominator recovered into the partition axis by a transpose of the same
  block, then a per-partition reciprocal multiply normalizes.

q and k are staged host-side in [d, s] (transposed) layout — a pure layout
transform like the head split — so the device spends no PE/DVE time
transposing inputs.  Score and O matmuls run as float32r (full-rate fp32,
~1.6e-4 component error).  Emission is software-pipelined across pairs: the
next pair's loads/projections and the previous chunk's epilogue are
interleaved into the exp-bound chunk loop so ScalarE (the softmax bottleneck)
never idles.  Dependencies follow emission order, so every producer piece is
emitted before its first consumer.
"""

import numpy as np

import concourse.bacc as bacc
import concourse.mybir as mybir
import concourse.tile as tile
from concourse.bass_utils import run_bass_kernel_spmd
from concourse.masks import make_identity

F32 = mybir.dt.float32
F32R = mybir.dt.float32r

N_CORES = 8
B, S, D_EMBED = 2, 2048, 1024
N_HEADS = 16
D = 64  # d_head
NPAIR = (B * N_HEADS) // N_CORES  # (b,h) pairs per core = 4
NT = S // 128  # 16 sk-tiles of 128
NCHUNK = S // 512  # 4 sq chunks of 512
GROUPS = [2] * 8  # sk-tiles per ACT exp instruction (= psum banks per tile);
# uniform groups of 2 allow triple-buffered score tiles (3 x 2 banks), which
# hides the cross-engine sem latency at chunk boundaries
MAXG = max(GROUPS)

_NC_CACHE = {}


def build_nc(npair=NPAIR, repeat=1):
    nc = bacc.Bacc("TRN2", target_bir_lowering=False)
    qs_d = nc.dram_tensor("qs", [npair, D, S], F32R, kind="ExternalInput").ap()
    ks_d = nc.dram_tensor("ks", [npair, D, S], F32R, kind="ExternalInput").ap()
    vs_d = nc.dram_tensor("vs", [npair, S, D + 1], F32R, kind="ExternalInput").ap()
    cst_d = nc.dram_tensor("cst", [D + 1, 2 * D + 1], F32R, kind="ExternalInput").ap()
    out_d = nc.dram_tensor("out", [npair, S, D], F32, kind="ExternalOutput").ap()

    with tile.TileContext(nc) as tc:
        with (
            tc.tile_pool(name="const", bufs=1) as const,
            tc.tile_pool(name="io", bufs=2) as io,
            tc.tile_pool(name="tr", bufs=2) as tr,
            tc.tile_pool(name="tr_ps", bufs=1, space="PSUM") as tr_ps,
            tc.tile_pool(name="pt", bufs=6) as pt_pool,
            tc.tile_pool(name="st_ps", bufs=3, space="PSUM") as st_ps,
            tc.tile_pool(name="oa_ps", bufs=1, space="PSUM") as oa_ps,
        ):
            ident = const.tile([128, 128], F32)
            make_identity(nc, ident)
            cst_sb = const.tile([D + 1, 2 * D + 1], F32R)
            m1_sb = cst_sb[0:D, 0:D]
            wv_sb = cst_sb[:, D : 2 * D + 1]

            def load_consts():
                nc.sync.dma_start(out=cst_sb, in_=cst_d)

            pairs = {}  # p -> dict of live tiles
            rep = 0

            def alloc_pair(p):
                st = {}
                st["khT"] = tr.tile([D, S], F32R, tag="khT", name=f"khT{rep}_{p}")
                st["qhT"] = tr.tile([D, S], F32R, tag="qhT", name=f"qhT{rep}_{p}")
                st["qtT"] = tr.tile([D, S], F32R, tag="qtT", name=f"qtT{rep}_{p}")
                st["v_sb"] = io.tile(
                    [128, NT, D + 1], F32R, tag="v_sb", name=f"v_sb{rep}_{p}"
                )
                pairs[p] = st

            def load_kq(p, quarter, vh=None):
                """DMA a [64, 512] quarter of khT+qhT; optionally a v half."""

                def run():
                    st = pairs[p]
                    sl = slice(quarter * 512, (quarter + 1) * 512)
                    nc.sync.dma_start(out=st["khT"][:, sl], in_=ks_d[p][:, sl])
                    nc.sync.dma_start(out=st["qhT"][:, sl], in_=qs_d[p][:, sl])
                    if vh is not None:
                        h = slice(vh * 8, vh * 8 + 8)
                        nc.sync.dma_start(
                            out=st["v_sb"][:, h, :],
                            in_=vs_d[p].rearrange("(t r) d -> r t d", r=128)[:, h, :],
                        )

                return run

            def qproj(p, c):
                """qtT chunk c = M1^T @ qhT chunk c."""

                def run():
                    st = pairs[p]
                    pj_ps = tr_ps.tile(
                        [D, 512], F32, tag="t_ps", name=f"pj{rep}_{p}_{c}"
                    )
                    nc.tensor.matmul(
                        pj_ps,
                        m1_sb,
                        st["qhT"][:, c * 512 : (c + 1) * 512],
                        start=True,
                        stop=True,
                    )
                    nc.vector.tensor_copy(
                        st["qtT"][:, c * 512 : (c + 1) * 512], pj_ps
                    )

                return run

            def E_pieces(p, c, final=False):
                """Epilogue for chunk c of pair p: project by Wv (denominator
                rides along), transpose back, normalize, store chunk."""
                st = pairs[p]

                def proj():
                    st[f"out{c}"] = io.tile(
                        [128, 4, D], F32, tag="out_sb", name=f"out_sb{rep}_{p}_{c}"
                    )

                def norm(tt0):
                    def run():
                        for tt in (tt0, tt0 + 1):
                            cols = slice(c * 512 + tt * 128, c * 512 + tt * 128 + 128)
                            if final:
                                ot = st_ps.tile(
                                    [128, MAXG * 512],
                                    F32,
                                    tag="st",
                                    name=f"ot{rep}_{p}_{c}_{tt}",
                                )
                            else:
                                ot = tr_ps.tile(
                                    [128, 512],
                                    F32,
                                    tag="t_ps",
                                    name=f"ot{rep}_{p}_{c}_{tt}",
                                )
                            # out block = O1 @ Wv^T straight from the
                            # transposed accumulator (oaug as lhsT)
                            nc.tensor.matmul(
                                ot[:, 0:D],
                                st["oaug"][0:D, cols],
                                wv_sb[0:D, 0:D],
                                start=True,
                                stop=True,
                            )
                            # denominator rides over via a transpose of the
                            # same oaug block (only column D is used)
                            nc.tensor.transpose(
                                ot[:, 128 : 128 + D + 1],
                                st["oaug"][:, cols].bitcast(F32),
                                ident[0 : D + 1, 0 : D + 1],
                            )
                            recip = tr.tile(
                                [128, 1],
                                F32,
                                tag="recip",
                                name=f"rc{rep}_{p}_{c}_{tt}",
                            )
                            nc.vector.reciprocal(
                                recip, ot[:, 128 + D : 128 + D + 1]
                            )
                            nc.vector.tensor_scalar_mul(
                                st[f"out{c}"][:, tt, :],
                                in0=ot[:, 0:D],
                                scalar1=recip,
                            )
                        if final:
                            sl = slice(c * 4 + tt0, c * 4 + tt0 + 2)
                            nc.sync.dma_start(
                                out=out_d[p].rearrange("(t r) d -> r t d", r=128)[
                                    :, sl, :
                                ],
                                in_=st[f"out{c}"][:, tt0 : tt0 + 2, :],
                            )
                        elif tt0 + 2 == 4:
                            sl = slice(c * 4, c * 4 + 4)
                            nc.sync.dma_start(
                                out=out_d[p].rearrange("(t r) d -> r t d", r=128)[
                                    :, sl, :
                                ],
                                in_=st[f"out{c}"],
                            )

                    return run

                return [proj, norm(0), norm(2)]

            pending = {}  # (p, c) -> pre-issued group-0 score tile

            def issue_g0(p, c):
                st_p = pairs[p]
                st = st_ps.tile(
                    [128, MAXG * 512], F32, tag="st", name=f"st0_{rep}_{p}_{c}"
                )
                for j in range(GROUPS[0]):
                    nc.tensor.matmul(
                        st[:, j * 512 : (j + 1) * 512],
                        st_p["khT"][:, j * 128 : (j + 1) * 128],
                        st_p["qtT"][:, c * 512 : (c + 1) * 512],
                        start=True,
                        stop=True,
                    )
                pending[(p, c)] = st

            def emit_C_chunk(p, c, fillers, next_key=None, final=False):
                st_p = pairs[p]
                if c == 0:
                    st_p["oaug"] = tr.tile(
                        [D + 1, S], F32R, tag="oaug", name=f"oaug{rep}_{p}"
                    )
                oa = oa_ps.tile([D + 1, 512], F32, tag="oa", name=f"oa{rep}_{p}_{c}")
                fi = 0
                sk0 = 0
                for gi, gsz in enumerate(GROUPS):
                    if gi == 0 and (p, c) in pending:
                        st = pending.pop((p, c))
                    else:
                        st = st_ps.tile([128, MAXG * 512], F32, tag="st")
                        for j in range(gsz):
                            sk = sk0 + j
                            nc.tensor.matmul(
                                st[:, j * 512 : (j + 1) * 512],
                                st_p["khT"][:, sk * 128 : (sk + 1) * 128],
                                st_p["qtT"][:, c * 512 : (c + 1) * 512],
                                start=True,
                                stop=True,
                            )
                    if gi == len(GROUPS) - 1 and next_key is not None:
                        # pre-issue the next chunk's group 0 so its scores are
                        # ready (and the sem visible) before our exps drain
                        issue_g0(*next_key)
                    ptile = pt_pool.tile([128, MAXG * 512], F32R, tag="pt")
                    nc.scalar.activation(
                        ptile[:, 0 : gsz * 512],
                        st[:, 0 : gsz * 512],
                        mybir.ActivationFunctionType.Exp,
                    )
                    for j in range(gsz):
                        sk = sk0 + j
                        nc.tensor.matmul(
                            oa,
                            st_p["v_sb"][:, sk, :],
                            ptile[:, j * 512 : (j + 1) * 512],
                            start=(sk == 0),
                            stop=(sk == NT - 1),
                        )
                    sk0 += gsz
                    if gi >= 1 and fi < len(fillers):
                        fillers[fi]()
                        fi += 1
                if final:
                    for h in (0, 1):
                        nc.vector.tensor_copy(
                            st_p["oaug"][:, c * 512 + h * 256 : c * 512 + h * 256 + 256],
                            oa[:, h * 256 : h * 256 + 256],
                        )
                else:
                    nc.vector.tensor_copy(
                        st_p["oaug"][:, c * 512 : (c + 1) * 512], oa
                    )
                while fi < len(fillers):
                    fillers[fi]()
                    fi += 1

            # ---- software pipeline over pairs ----
            for rep in range(repeat):
                alloc_pair(0)
                if rep == 0:
                    # warm the PE clock gate (HAM) while the first loads are
                    # in flight, so the first real matmuls run at full rate
                    for w in range(4):
                        warm_ps = tr_ps.tile(
                            [128, 128], F32, tag="t_ps", name=f"warm{w}"
                        )
                        nc.tensor.transpose(warm_ps, ident, ident)
                st0 = pairs[0]
                # minimal data for the first score matmul + first exp:
                # qhT quarter 0 (for qtT chunk 0) and khT tiles sk=0,1
                nc.sync.dma_start(out=st0["qhT"][:, 0:512], in_=qs_d[0][:, 0:512])
                load_consts()
                nc.sync.dma_start(out=st0["khT"][:, 0:256], in_=ks_d[0][:, 0:256])
                qproj(0, 0)()
                nc.sync.dma_start(out=st0["khT"][:, 256:512], in_=ks_d[0][:, 256:512])
                nc.sync.dma_start(
                    out=st0["v_sb"][:, 0:4, :],
                    in_=vs_d[0].rearrange("(t r) d -> r t d", r=128)[:, 0:4, :],
                )
                load_kq(0, 1)()
                nc.sync.dma_start(
                    out=st0["v_sb"][:, 4:8, :],
                    in_=vs_d[0].rearrange("(t r) d -> r t d", r=128)[:, 4:8, :],
                )
                load_kq(0, 2)()

                def vq(lo):
                    def run():
                        nc.sync.dma_start(
                            out=st0["v_sb"][:, lo : lo + 4, :],
                            in_=vs_d[0].rearrange("(t r) d -> r t d", r=128)[
                                :, lo : lo + 4, :
                            ],
                        )

                    return run
                for p in range(npair):
                    nxt = p + 1 if p + 1 < npair else None
                    if nxt is not None and p == 0:
                        alloc_pair(nxt)
                    for c in range(NCHUNK):
                        if p == 0 and c == 0:
                            fillers = [
                                load_kq(0, 3),
                                vq(8),
                                vq(12),
                                qproj(0, 1),
                                qproj(0, 2),
                                qproj(0, 3),
                            ]
                            if nxt is not None:
                                fillers.append(load_kq(nxt, 0, vh=0))
                        else:
                            fillers = (
                                list(E_pieces(p, c - 1))
                                if c > 0
                                else list(E_pieces(p - 1, 3))
                            )
                            if c == 0:
                                fillers.append(qproj(p, 3))
                                if nxt is not None:
                                    if p > 0:
                                        alloc_pair(nxt)
                                    fillers.append(load_kq(nxt, 0, vh=0))
                            elif nxt is not None:
                                if c == 1:
                                    fillers += [
                                        load_kq(nxt, 1, vh=1),
                                        load_kq(nxt, 2),
                                    ]
                                elif c == 2:
                                    fillers += [load_kq(nxt, 3), qproj(nxt, 0)]
                                else:
                                    fillers += [qproj(nxt, 1), qproj(nxt, 2)]
                        if c < NCHUNK - 1:
                            nk = (p, c + 1)
                        elif p + 1 < npair:
                            nk = (p + 1, 0)
                        else:
                            nk = None
                        emit_C_chunk(
                            p, c, fillers, next_key=nk,
                            final=(nk is None),
                        )
                for piece in E_pieces(npair - 1, 3, final=True):
                    piece()
                pairs.clear()
    nc.finalize()
    return nc


def _host_prep(k, q, v, Wk, Wq, Wv):
    m1 = ((Wq.T @ Wk) / np.sqrt(np.float32(D))).astype(np.float32)
    wv_aug = np.zeros((D + 1, D + 1), dtype=np.float32)
    wv_aug[:D, :D] = Wv.T
    wv_aug[D, D] = 1.0

    # [B, S, E] -> heads split and transposed to [B*H, D, S]
    def split_heads_T(x):
        return np.ascontiguousarray(
            x.reshape(B, S, N_HEADS, D)
            .transpose(0, 2, 3, 1)
            .reshape(B * N_HEADS, D, S)
        )

    qhT = split_heads_T(q)
    khT = split_heads_T(k)
    vh = v.reshape(B, S, N_HEADS, D).transpose(0, 2, 1, 3).reshape(B * N_HEADS, S, D)
    vh_aug = np.empty((B * N_HEADS, S, D + 1), dtype=np.float32)
    vh_aug[:, :, :D] = vh
    vh_aug[:, :, D] = 1.0
    return qhT, khT, vh_aug, m1, wv_aug


def kernel(k, q, v, Wk, Wq, Wv):
    k = np.asarray(k, dtype=np.float32)
    q = np.asarray(q, dtype=np.float32)
    v = np.asarray(v, dtype=np.float32)
    Wk = np.asarray(Wk, dtype=np.float32)
    Wq = np.asarray(Wq, dtype=np.float32)
    Wv = np.asarray(Wv, dtype=np.float32)

    qhT, khT, vh_aug, m1, wv_aug = _host_prep(k, q, v, Wk, Wq, Wv)
    cst = np.zeros((D + 1, 2 * D + 1), dtype=np.float32)
    cst[0:D, 0:D] = m1
    cst[:, D : 2 * D + 1] = wv_aug

    if "nc" not in _NC_CACHE:
        _NC_CACHE["nc"] = build_nc()
    nc = _NC_CACHE["nc"]

    in_maps = []
    for c in range(N_CORES):
        sl = slice(c * NPAIR, (c + 1) * NPAIR)
        in_maps.append(
            {
                "qs": qhT[sl],
                "ks": khT[sl],
                "vs": vh_aug[sl],
                "cst": cst,
            }
        )

    res = run_bass_kernel_spmd(nc, in_maps, core_ids=list(range(N_CORES)))
    outs = np.stack([r["out"] for r in res.results])  # [8, NPAIR, S, D]
    out = outs.reshape(B, N_HEADS, S, D).transpose(0, 2, 1, 3).reshape(B, S, D_EMBED)
    return out

